# revision 69
# baseline (speedup 1.0000x reference)
"""GQA attention kernel for 8 Trainium2 NeuronCores.

Sharding: core = (batch b, kv_group g), b in {0,1}, g in {0..3}.
Each core computes the 4 heads of one KV group for one batch and the
partial output projection for those heads; the host sums the 4 group
partials per batch.  Zero duplicated compute across cores.

Fully fused single-pipeline design (v2):
  - every matmul operand is bf16: FWL + background weight buffer hide
    LDWEIGHTS, DMA and SBUF halve; PSUM accumulation stays fp32.
    Measured end-to-end max rel err ~6e-3 vs the 2e-2 gate.
  - one flat instruction stream: K/V/Q(h0) projections run up front,
    the remaining Q projections and the whole output projection are
    emitted as "side units" interleaved between attention tiles, so
    the PE never idles at phase boundaries and the ~55us output
    projection largely hides under the ACT/DVE-bound attention loop.
  - softmax sums off the PE: probs tiles accumulate on DVE in bf16
    (two independent 8-deep chains), then one tiny 4-matmul
    partition-reduce; normalization is a single GpSimd divide.
  - per-e-tile-grouped DMAs ordered so the first projection matmul
    starts as soon as ~0.6 MiB has landed.
"""

import numpy as np

# problem shape (hardcoded per contract)
B, S, E = 2, 2048, 2048
H, G, D = 16, 4, 128
R = H // G          # heads per kv group = 4
ST = S // 128       # 16 t-tiles
ET = E // 128       # 16 e-tiles
SC = S // 512       # 4 s-chunks
NPAIR = S // 1024   # 2 q-chunk pairs

_cache = {}


def _split_multi_waits(nc, maxw=1):
    """Walrus in this container accepts only one sync-wait per
    instruction; move extra waits onto preceding same-engine NoOps."""
    from concourse import mybir

    n_split = 0
    for fn in nc.m.functions:
        for bb in fn.blocks:
            out = []
            changed = False
            for inst in bb.instructions:
                si = inst.sync_info
                waits = list(si.on_wait or []) if si is not None else []
                if len(waits) > maxw:
                    changed = True
                    n_split += 1
                    head, tail = waits[:-maxw], waits[-maxw:]
                    for j in range(0, len(head), maxw):
                        nop = mybir.InstNoOp(
                            name=f"{inst.name}-wsplit{j}", ins=[], outs=[]
                        )
                        nop.engine = inst.engine
                        nop.sync_info = mybir.SyncInfo(
                            on_wait=head[j : j + maxw], on_update=[]
                        )
                        out.append(nop)
                    si.on_wait = tail
                out.append(inst)
            if changed:
                bb.instructions = out
    return n_split


def _build_program():
    import contextlib

    import concourse.bass as bass
    import concourse.tile as tile
    from concourse import mybir
    from concourse.masks import make_identity

    BF16 = mybir.dt.bfloat16
    F32 = mybir.dt.float32
    Exp = mybir.ActivationFunctionType.Exp
    Add = mybir.AluOpType.add
    Mult = mybir.AluOpType.mult

    nc = bass.Bass(target_bir_lowering=False)

    # inputs arrive pre-tiled by the host into the exact SBUF image
    # (partition-major), so every DMA is long contiguous runs: one
    # trigger, 128 descriptors, full bandwidth
    xd = nc.dram_tensor("xd", [128, SC, ET, 512], BF16, kind="ExternalInput")
    wq = nc.dram_tensor("wq", [128, R, ET, D], BF16, kind="ExternalInput")
    wk = nc.dram_tensor("wk", [128, ET, D], BF16, kind="ExternalInput")
    wv = nc.dram_tensor("wv", [128, ET, D], BF16, kind="ExternalInput")
    wo = nc.dram_tensor("wo", [128, R, E], BF16, kind="ExternalInput")
    bqv = nc.dram_tensor("bqv", [R * D], F32, kind="ExternalInput")
    bkv = nc.dram_tensor("bkv", [D], F32, kind="ExternalInput")
    bvv = nc.dram_tensor("bvv", [D], F32, kind="ExternalInput")
    otd = nc.dram_tensor("ot", [E, S], BF16, kind="ExternalOutput")

    with tile.TileContext(nc) as tc:
        with contextlib.ExitStack() as ctx:
            ep = ctx.enter_context
            consts = ep(tc.tile_pool(name="consts", bufs=1))
            main = ep(tc.tile_pool(name="main", bufs=1))
            probs_pool = ep(tc.tile_pool(name="probs", bufs=8))
            accp = ep(tc.tile_pool(name="accp", bufs=2))
            normp = ep(tc.tile_pool(name="normp", bufs=2))
            ostage = ep(tc.tile_pool(name="ostage", bufs=6))
            psP = ep(tc.tile_pool(name="psP", bufs=2, space="PSUM"))
            # attention-phase PSUM pools live in their own stack so the
            # final drain can close them and claim the banks
            attn_stack = ctx.enter_context(contextlib.ExitStack())
            psS = attn_stack.enter_context(
                tc.tile_pool(name="psS", bufs=2, space="PSUM")
            )
            psA = attn_stack.enter_context(
                tc.tile_pool(name="psA", bufs=1, space="PSUM")
            )

            ident_f = consts.tile([128, 128], F32)
            make_identity(nc, ident_f)
            ident = consts.tile([128, 128], BF16)
            nc.vector.tensor_copy(ident, ident_f)
            ones_f = consts.tile([128, 128], F32)
            nc.gpsimd.memset(ones_f, 1.0)
            ones = consts.tile([128, 128], BF16)
            nc.vector.tensor_copy(ones, ones_f)
            bq_sb = consts.tile([128, R], F32)
            nc.sync.dma_start(bq_sb, bqv.rearrange("(o p) -> p o", p=128))
            bk_sb = consts.tile([128, 1], F32)
            nc.sync.dma_start(bk_sb, bkv.rearrange("(o p) -> p o", p=128))
            bv_sb = consts.tile([128, 1], F32)
            nc.sync.dma_start(bv_sb, bvv.rearrange("(o p) -> p o", p=128))

            QT = main.tile([128, R, S], BF16)    # QT[d, h, s]
            KT = main.tile([128, S], BF16)       # KT[d, t]
            V = main.tile([128, ST, D], BF16)    # V[t%128, tt, d]
            VT = main.tile([128, S], BF16)
            outT = main.tile([128, R, S], BF16)  # normalized attn out
            wk_sb = main.tile([128, ET, D], BF16)
            wv_sb = main.tile([128, ET, D], BF16)
            wq_sb = main.tile([128, R, ET, D], BF16)
            wo_sb = main.tile([128, R, E], BF16)
            xtiles = [
                main.tile([128, ET, 512], BF16, name=f"xtile{sc}")
                for sc in range(SC)
            ]

            # DMA trigger order is the startup critical path: K weights and
            # the first x chunks go first so the PE starts early; x chunks
            # arrive in the order the upfront K/V units consume them
            # ALL input loads ride ONE queue (Scalar: shortest preamble), in
            # exact consumption order - a single queue still fans out over
            # all 16 DMA engines, and FIFO order means chunks complete in
            # the order the projection units consume them (aggregate HBM
            # pull is ~200GB/s with all 8 cores active, so order is king)
            nc.scalar.dma_start(wk_sb, wk[:, :, :])
            nc.scalar.dma_start(xtiles[0][:, 0:8], xd[:, 0, 0:8])
            nc.scalar.dma_start(xtiles[0][:, 8:16], xd[:, 0, 8:16])
            nc.scalar.dma_start(wv_sb, wv[:, :, :])
            nc.scalar.dma_start(xtiles[1][:, 0:8], xd[:, 1, 0:8])
            nc.scalar.dma_start(xtiles[1][:, 8:16], xd[:, 1, 8:16])
            nc.scalar.dma_start(xtiles[2][:, 0:8], xd[:, 2, 0:8])
            nc.scalar.dma_start(xtiles[2][:, 8:16], xd[:, 2, 8:16])
            nc.scalar.dma_start(wq_sb[:, 0:1], wq[:, 0:1])
            nc.scalar.dma_start(xtiles[3][:, 0:8], xd[:, 3, 0:8])
            nc.scalar.dma_start(xtiles[3][:, 8:16], xd[:, 3, 8:16])
            nc.scalar.dma_start(wq_sb[:, 1:4], wq[:, 1:4])

            # ---------- work units ----------
            pending_tr = []

            def flush_tr():
                while pending_tr:
                    tt = pending_tr.pop(0)
                    psv = psP.tile([128, 128], BF16, tag="p1", name="psv")
                    nc.tensor.transpose(
                        psv, VT[:, tt * 128 : (tt + 1) * 128], ident
                    )
                    nc.vector.tensor_copy(V[:, tt], psv)

            def unit_proj(kind, sc, h=0, defer_tr=False):
                cs = slice(sc * 512, (sc + 1) * 512)
                flush_tr()
                psum = psP.tile([128, 512], F32, tag="p1", name="psum")
                for e in range(ET):
                    if kind == "k":
                        lhsT = wk_sb[:, e]
                    elif kind == "v":
                        lhsT = wv_sb[:, e]
                    else:
                        lhsT = wq_sb[:, h, e]
                    nc.tensor.matmul(
                        psum, lhsT, xtiles[sc][:, e],
                        start=(e == 0), stop=(e == ET - 1),
                    )
                if kind == "k":
                    nc.scalar.add(KT[:, cs], psum, bk_sb[:, 0:1])
                elif kind == "v":
                    nc.scalar.add(VT[:, cs], psum, bv_sb[:, 0:1])
                    pending_tr.extend(sc * 4 + q for q in range(4))
                    if not defer_tr:
                        flush_tr()
                else:
                    nc.scalar.add(QT[:, h, cs], psum, bq_sb[:, h : h + 1])

            cfg = {"p3pool": psP}

            def unit_p3(et, pr, pool=None):
                # both 512-col halves of a q-pair for this e-tile row, then
                # ONE output DMA of [128, 1024] (fewer triggers, longer runs)
                pool = pool or cfg["p3pool"]
                st = ostage.tile([128, 1024], BF16, tag="ost", name="st")
                for half in range(2):
                    sc = pr * 2 + half
                    ps = pool.tile([128, 512], F32, tag="p1", name="ps3")
                    for h in range(R):
                        nc.tensor.matmul(
                            ps,
                            wo_sb[:, h, et * 128 : (et + 1) * 128],
                            outT[:, h, sc * 512 : (sc + 1) * 512],
                            start=(h == 0), stop=(h == R - 1),
                        )
                    nc.vector.tensor_copy(
                        st[:, half * 512 : (half + 1) * 512], ps
                    )
                # alternate output triggers across two queues so the final
                # transfers don't serialize behind one trigger stream
                eng = nc.gpsimd if (et + pr) % 2 == 0 else nc.scalar
                eng.dma_start(
                    otd[et * 128 : (et + 1) * 128,
                        pr * 1024 : (pr + 1) * 1024],
                    st,
                )

            side = []

            def pump(n):
                for _ in range(n):
                    if side:
                        side.pop(0)()

            # ---------- upfront projections (interleaved K/V per chunk to
            # match x chunk arrival order; chunk 3's K/V become the first
            # side units so iter0 starts as soon as wq(h0) lands) ----------
            for sc in range(3):
                unit_proj("k", sc)
                unit_proj("v", sc, defer_tr=True)
            unit_proj("q", 0, 0)
            unit_proj("q", 1, 0)
            flush_tr()
            # wo is only needed once output-projection side units start
            # (~100us in): trigger from the now-quiet Sync queue so its 2MiB
            # doesn't compete with the startup x loads
            nc.sync.dma_start(wo_sb, wo[:, :, :])

            # remaining projections stream in as side work, ordered by when
            # the attention iterations consume them: K/V(s3) are needed from
            # iter0 tt=12 on; pr0 iters need Q(h,0),(h,1); pr1 (h,2),(h,3)
            side.append(lambda: unit_proj("k", 3))
            side.append(lambda: unit_proj("v", 3))
            for h in range(1, R):
                side.append(lambda h=h: unit_proj("q", 0, h))
                side.append(lambda h=h: unit_proj("q", 1, h))
            for h in range(R):
                side.append(lambda h=h: unit_proj("q", 2, h))
                side.append(lambda h=h: unit_proj("q", 3, h))

            # ---------- attention + interleaved side units ----------
            def mm_scores(pss, h, q0, tt):
                kslice = KT[:, tt * 128 : (tt + 1) * 128]
                for hf in range(2):
                    nc.tensor.matmul(
                        pss[:, hf * 512 : (hf + 1) * 512],
                        kslice,
                        QT[:, h, q0 + hf * 512 : q0 + (hf + 1) * 512],
                        start=True, stop=True,
                    )

            iters = [(pr, h) for pr in range(NPAIR) for h in range(R)]
            deferred = []  # (slot, closure): recip/mult of the PREVIOUS iter

            def flush_deferred(slot):
                while deferred and deferred[0][0] <= slot:
                    deferred.pop(0)[1]()

            for it, (pr, h) in enumerate(iters):
                q0 = pr * 1024
                out_ps = psA.tile([128, 1024], F32, tag="av", name="out_ps")
                pss_tiles = [None, None]
                pss_tiles[0] = psS.tile([128, 1024], F32, tag="sc", name="pss")
                mm_scores(pss_tiles[0], h, q0, 0)
                acc_a = accp.tile([128, 1024], BF16, tag="acca", name="acc_a")
                acc_b = accp.tile([128, 1024], BF16, tag="accb", name="acc_b")
                for tt in range(ST):
                    pt = probs_pool.tile([128, 1024], BF16, tag="pb", name="pt")
                    nc.scalar.activation(pt, pss_tiles[tt % 2], Exp)
                    # keep independent PE work queued ahead of the
                    # exp-gated AV matmuls
                    if tt + 1 < ST:
                        pss_tiles[(tt + 1) % 2] = psS.tile(
                            [128, 1024], F32, tag="sc", name="pss"
                        )
                        mm_scores(pss_tiles[(tt + 1) % 2], h, q0, tt + 1)
                    for hf in range(2):
                        hs = slice(hf * 512, (hf + 1) * 512)
                        nc.tensor.matmul(
                            out_ps[:, hs], V[:, tt], pt[:, hs],
                            start=(tt == 0), stop=(tt == ST - 1),
                        )
                    # softmax denominators: bf16 elementwise accumulation
                    # on DVE (two 8-deep chains), off the PE entirely
                    if tt == 0:
                        nc.vector.tensor_copy(acc_a, pt)
                    elif tt == 1:
                        nc.vector.tensor_copy(acc_b, pt)
                    elif tt % 2 == 0:
                        nc.vector.tensor_tensor(acc_a, acc_a, pt, Add)
                    else:
                        nc.vector.tensor_tensor(acc_b, acc_b, pt, Add)
                    # previous iter's slow reciprocal runs HERE, mid-iter,
                    # where the in-order DVE queue has slack - never at an
                    # iteration boundary where it would gate probs recycling
                    flush_deferred(tt)
                    # iter0 must emit K/V(s3) before the attention matmuls
                    # that consume them (in-order PE queue: consumer-first
                    # would deadlock), and early enough to hide x3 landing
                    if (
                        tt in (7, 15)
                        or (it >= 4 and tt in (3, 11))
                        or (it == 0 and tt in (5, 9, 11))
                    ):
                        pump(1)
                # partition-reduce the two chain accumulators: 4 small
                # matmuls -> sums replicated across partitions
                sums_ps = psS.tile([128, 1024], F32, tag="sc", name="sums_ps")
                for ai, acc in enumerate((acc_a, acc_b)):
                    for hf in range(2):
                        hs = slice(hf * 512, (hf + 1) * 512)
                        nc.tensor.matmul(
                            sums_ps[:, hs], ones, acc[:, hs],
                            start=(ai == 0), stop=(ai == 1),
                        )
                av_sb = normp.tile([128, 1024], BF16, tag="a", name="av_sb")
                nc.vector.tensor_copy(av_sb, out_ps)
                sums_sb = normp.tile([128, 1024], F32, tag="s", name="sums_sb")
                nc.vector.tensor_copy(sums_sb, sums_ps)

                # the ~6.5us DVE reciprocal would starve probs recycling if
                # run whole: split into 4 chunks spread between next-iter
                # adds, with the normalize multiply trailing
                rc = normp.tile([128, 1024], BF16, tag="r", name="rc")

                def rchunk(c, sums_sb=sums_sb, rc=rc):
                    cs4 = slice(c * 256, (c + 1) * 256)
                    with nc.allow_low_precision(
                        reason="bf16 softmax scale, ~0.4% ok at 2e-2 gate"
                    ):
                        nc.vector.reciprocal(rc[:, cs4], sums_sb[:, cs4])

                def fmult(h=h, q0=q0, av_sb=av_sb, rc=rc):
                    # all-bf16 multiply on the otherwise-idle GpSimd engine
                    nc.gpsimd.tensor_tensor(
                        outT[:, h, q0 : q0 + 1024], av_sb, rc, Mult
                    )

                for c in range(4):
                    deferred.append((3 + 2 * c, lambda c=c: rchunk(c)))
                deferred.append((11, fmult))
                if it == 3:
                    # pr0 fully normalized soon: its output projection
                    # columns become available side work
                    for et in range(ET):
                        side.append(lambda et=et: unit_p3(et, 0))
            # final drain: release the attention PSUM banks and run the
            # remaining output-projection rows with deep buffering
            attn_stack.close()
            psD = ep(tc.tile_pool(name="psD", bufs=6, space="PSUM"))
            cfg["p3pool"] = psD
            leftovers = list(side)
            side.clear()
            # interleave the last iteration's deferred normalize between the
            # first drain units so it never head-blocks their evacuations
            for i, fn in enumerate(leftovers):
                fn()
                flush_deferred(3 + 2 * i)
            flush_deferred(ST)
            for et in range(ET):
                unit_p3(et, 1)

    _split_multi_waits(nc)
    return nc


def _prepare(x, Wq, bq, Wk, bk, Wv, bv, Wo, bo):
    """Host-side sharding: build per-core input maps (bf16 operands)."""
    import ml_dtypes

    bf16 = ml_dtypes.bfloat16
    x = np.asarray(x, dtype=np.float32)
    Wq = np.asarray(Wq, dtype=np.float32)
    bq = np.asarray(bq, dtype=np.float32)
    Wk = np.asarray(Wk, dtype=np.float32)
    bk = np.asarray(bk, dtype=np.float32)
    Wv = np.asarray(Wv, dtype=np.float32)
    bv = np.asarray(bv, dtype=np.float32)
    Wo = np.asarray(Wo, dtype=np.float32)

    isd = np.float32(1.0 / np.sqrt(D))

    # pre-tile everything into the partition-major SBUF images the kernel
    # DMAs verbatim: contiguous per-partition rows = minimal descriptors
    def tile_w(w):  # [E, M] -> [128, ET, M]
        m = w.shape[1]
        return np.ascontiguousarray(
            w.reshape(ET, 128, m).transpose(1, 0, 2)
        ).astype(bf16)

    xds = [
        np.ascontiguousarray(
            x[b].T.reshape(ET, 128, SC, 512).transpose(1, 2, 0, 3)
        ).astype(bf16)
        for b in range(B)
    ]
    # wq is head-major [128, H, ET, D] so the kernel can pull head 0 early
    Wq_s = np.ascontiguousarray(
        (Wq * isd).reshape(ET, 128, H, D).transpose(1, 2, 0, 3)
    ).astype(bf16)
    Wk_s = tile_w(Wk)
    Wv_s = tile_w(Wv)
    Wo_t = np.ascontiguousarray(
        Wo.reshape(G, R, 128, E).transpose(2, 0, 1, 3)
    ).astype(bf16)  # [128, G, R, E]
    in_maps = []
    for core in range(8):
        b, g = divmod(core, G)
        in_maps.append({
            "xd": xds[b],
            "wq": np.ascontiguousarray(Wq_s[:, g * R : (g + 1) * R]),
            "wk": np.ascontiguousarray(Wk_s[:, :, g * D : (g + 1) * D]),
            "wv": np.ascontiguousarray(Wv_s[:, :, g * D : (g + 1) * D]),
            "wo": np.ascontiguousarray(Wo_t[:, g]),
            "bqv": bq[g * R * D : (g + 1) * R * D] * isd,
            "bkv": bk[g * D : (g + 1) * D],
            "bvv": bv[g * D : (g + 1) * D],
        })
    return in_maps


def _gather(results, bo):
    bo = np.asarray(bo, dtype=np.float32)
    out = np.empty((B, S, E), dtype=np.float32)
    for b in range(B):
        acc = results[b * G]["ot"].astype(np.float32)
        for g in range(1, G):
            acc += results[b * G + g]["ot"].astype(np.float32)
        out[b] = acc.T + bo
    return out


def kernel(x, Wq, bq, Wk, bk, Wv, bv, Wo, bo):
    from concourse.bass_utils import run_bass_kernel_spmd

    if "nc" not in _cache:
        _cache["nc"] = _build_program()
    nc = _cache["nc"]
    in_maps = _prepare(x, Wq, bq, Wk, bk, Wv, bv, Wo, bo)
    res = run_bass_kernel_spmd(nc, in_maps, core_ids=list(range(8)))
    return _gather(res.results, bo)


# revision 71
# speedup vs baseline: 1.0214x; 1.0214x over previous
"""GQA attention kernel for 8 Trainium2 NeuronCores.

Sharding: core = (batch b, kv_group g), b in {0,1}, g in {0..3}.
Each core computes the 4 heads of one KV group for one batch and the
partial output projection for those heads; the host sums the 4 group
partials per batch.  Zero duplicated compute across cores.

Fully fused single-pipeline design (v2):
  - every matmul operand is bf16: FWL + background weight buffer hide
    LDWEIGHTS, DMA and SBUF halve; PSUM accumulation stays fp32.
    Measured end-to-end max rel err ~6e-3 vs the 2e-2 gate.
  - one flat instruction stream: K/V/Q(h0) projections run up front,
    the remaining Q projections and the whole output projection are
    emitted as "side units" interleaved between attention tiles, so
    the PE never idles at phase boundaries and the ~55us output
    projection largely hides under the ACT/DVE-bound attention loop.
  - softmax sums off the PE: probs tiles accumulate on DVE in bf16
    (two independent 8-deep chains), then one tiny 4-matmul
    partition-reduce; normalization is a single GpSimd divide.
  - per-e-tile-grouped DMAs ordered so the first projection matmul
    starts as soon as ~0.6 MiB has landed.
"""

import numpy as np

# problem shape (hardcoded per contract)
B, S, E = 2, 2048, 2048
H, G, D = 16, 4, 128
R = H // G          # heads per kv group = 4
ST = S // 128       # 16 t-tiles
ET = E // 128       # 16 e-tiles
SC = S // 512       # 4 s-chunks
NPAIR = S // 1024   # 2 q-chunk pairs

_cache = {}


def _split_multi_waits(nc, maxw=1):
    """Walrus in this container accepts only one sync-wait per
    instruction; move extra waits onto preceding same-engine NoOps."""
    from concourse import mybir

    n_split = 0
    for fn in nc.m.functions:
        for bb in fn.blocks:
            out = []
            changed = False
            for inst in bb.instructions:
                si = inst.sync_info
                waits = list(si.on_wait or []) if si is not None else []
                if len(waits) > maxw:
                    changed = True
                    n_split += 1
                    head, tail = waits[:-maxw], waits[-maxw:]
                    for j in range(0, len(head), maxw):
                        nop = mybir.InstNoOp(
                            name=f"{inst.name}-wsplit{j}", ins=[], outs=[]
                        )
                        nop.engine = inst.engine
                        nop.sync_info = mybir.SyncInfo(
                            on_wait=head[j : j + maxw], on_update=[]
                        )
                        out.append(nop)
                    si.on_wait = tail
                out.append(inst)
            if changed:
                bb.instructions = out
    return n_split


def _build_program():
    import contextlib

    import concourse.bass as bass
    import concourse.tile as tile
    from concourse import mybir
    from concourse.masks import make_identity

    BF16 = mybir.dt.bfloat16
    F32 = mybir.dt.float32
    Exp = mybir.ActivationFunctionType.Exp
    Add = mybir.AluOpType.add
    Mult = mybir.AluOpType.mult

    nc = bass.Bass(target_bir_lowering=False)

    # inputs arrive pre-tiled by the host into the exact SBUF image
    # (partition-major), so every DMA is long contiguous runs: one
    # trigger, 128 descriptors, full bandwidth
    xd = nc.dram_tensor("xd", [128, SC, ET, 512], BF16, kind="ExternalInput")
    wq = nc.dram_tensor("wq", [128, R, ET, D], BF16, kind="ExternalInput")
    wk = nc.dram_tensor("wk", [128, ET, D], BF16, kind="ExternalInput")
    wv = nc.dram_tensor("wv", [128, ET, D], BF16, kind="ExternalInput")
    wo = nc.dram_tensor("wo", [128, R, E], BF16, kind="ExternalInput")
    bqv = nc.dram_tensor("bqv", [R * D], F32, kind="ExternalInput")
    bkv = nc.dram_tensor("bkv", [D], F32, kind="ExternalInput")
    bvv = nc.dram_tensor("bvv", [D], F32, kind="ExternalInput")
    otd = nc.dram_tensor("ot", [E, S], BF16, kind="ExternalOutput")

    with tile.TileContext(nc) as tc:
        with contextlib.ExitStack() as ctx:
            ep = ctx.enter_context
            consts = ep(tc.tile_pool(name="consts", bufs=1))
            main = ep(tc.tile_pool(name="main", bufs=1))
            probs_pool = ep(tc.tile_pool(name="probs", bufs=8))
            accp = ep(tc.tile_pool(name="accp", bufs=3))
            normp = ep(tc.tile_pool(name="normp", bufs=2))
            ostage = ep(tc.tile_pool(name="ostage", bufs=6))
            psP = ep(tc.tile_pool(name="psP", bufs=2, space="PSUM"))
            # attention-phase PSUM pools live in their own stack so the
            # final drain can close them and claim the banks
            attn_stack = ctx.enter_context(contextlib.ExitStack())
            psS = attn_stack.enter_context(
                tc.tile_pool(name="psS", bufs=2, space="PSUM")
            )
            psA = attn_stack.enter_context(
                tc.tile_pool(name="psA", bufs=1, space="PSUM")
            )

            ident_f = consts.tile([128, 128], F32)
            make_identity(nc, ident_f)
            ident = consts.tile([128, 128], BF16)
            nc.vector.tensor_copy(ident, ident_f)
            ones_f = consts.tile([128, 128], F32)
            nc.gpsimd.memset(ones_f, 1.0)
            ones = consts.tile([128, 128], BF16)
            nc.vector.tensor_copy(ones, ones_f)
            bq_sb = consts.tile([128, R], F32)
            nc.sync.dma_start(bq_sb, bqv.rearrange("(o p) -> p o", p=128))
            bk_sb = consts.tile([128, 1], F32)
            nc.sync.dma_start(bk_sb, bkv.rearrange("(o p) -> p o", p=128))
            bv_sb = consts.tile([128, 1], F32)
            nc.sync.dma_start(bv_sb, bvv.rearrange("(o p) -> p o", p=128))

            QT = main.tile([128, R, S], BF16)    # QT[d, h, s]
            KT = main.tile([128, S], BF16)       # KT[d, t]
            V = main.tile([128, ST, D], BF16)    # V[t%128, tt, d]
            VT = main.tile([128, S], BF16)
            outT = main.tile([128, R, S], BF16)  # normalized attn out
            wk_sb = main.tile([128, ET, D], BF16)
            wv_sb = main.tile([128, ET, D], BF16)
            wq_sb = main.tile([128, R, ET, D], BF16)
            wo_sb = main.tile([128, R, E], BF16)
            xtiles = [
                main.tile([128, ET, 512], BF16, name=f"xtile{sc}")
                for sc in range(SC)
            ]

            # DMA trigger order is the startup critical path: K weights and
            # the first x chunks go first so the PE starts early; x chunks
            # arrive in the order the upfront K/V units consume them
            # ALL input loads ride ONE queue (Scalar: shortest preamble), in
            # exact consumption order - a single queue still fans out over
            # all 16 DMA engines, and FIFO order means chunks complete in
            # the order the projection units consume them (aggregate HBM
            # pull is ~200GB/s with all 8 cores active, so order is king)
            nc.scalar.dma_start(wk_sb, wk[:, :, :])
            nc.scalar.dma_start(xtiles[0][:, 0:8], xd[:, 0, 0:8])
            nc.scalar.dma_start(xtiles[0][:, 8:16], xd[:, 0, 8:16])
            nc.scalar.dma_start(wv_sb, wv[:, :, :])
            nc.scalar.dma_start(xtiles[1][:, 0:8], xd[:, 1, 0:8])
            nc.scalar.dma_start(xtiles[1][:, 8:16], xd[:, 1, 8:16])
            nc.scalar.dma_start(xtiles[2][:, 0:8], xd[:, 2, 0:8])
            nc.scalar.dma_start(xtiles[2][:, 8:16], xd[:, 2, 8:16])
            nc.scalar.dma_start(wq_sb[:, 0:1], wq[:, 0:1])
            nc.scalar.dma_start(xtiles[3][:, 0:8], xd[:, 3, 0:8])
            nc.scalar.dma_start(xtiles[3][:, 8:16], xd[:, 3, 8:16])
            nc.scalar.dma_start(wq_sb[:, 1:4], wq[:, 1:4])

            # ---------- work units ----------
            pending_tr = []

            def flush_tr():
                while pending_tr:
                    tt = pending_tr.pop(0)
                    psv = psP.tile([128, 128], BF16, tag="p1", name="psv")
                    nc.tensor.transpose(
                        psv, VT[:, tt * 128 : (tt + 1) * 128], ident
                    )
                    nc.vector.tensor_copy(V[:, tt], psv)

            def unit_proj(kind, sc, h=0, defer_tr=False):
                cs = slice(sc * 512, (sc + 1) * 512)
                flush_tr()
                psum = psP.tile([128, 512], F32, tag="p1", name="psum")
                for e in range(ET):
                    if kind == "k":
                        lhsT = wk_sb[:, e]
                    elif kind == "v":
                        lhsT = wv_sb[:, e]
                    else:
                        lhsT = wq_sb[:, h, e]
                    nc.tensor.matmul(
                        psum, lhsT, xtiles[sc][:, e],
                        start=(e == 0), stop=(e == ET - 1),
                    )
                if kind == "k":
                    nc.scalar.add(KT[:, cs], psum, bk_sb[:, 0:1])
                elif kind == "v":
                    nc.scalar.add(VT[:, cs], psum, bv_sb[:, 0:1])
                    pending_tr.extend(sc * 4 + q for q in range(4))
                    if not defer_tr:
                        flush_tr()
                else:
                    nc.scalar.add(QT[:, h, cs], psum, bq_sb[:, h : h + 1])

            cfg = {"p3pool": psP, "evac": nc.vector.tensor_copy}

            def unit_p3(et, pr, pool=None):
                # both 512-col halves of a q-pair for this e-tile row, then
                # ONE output DMA of [128, 1024] (fewer triggers, longer runs)
                pool = pool or cfg["p3pool"]
                st = ostage.tile([128, 1024], BF16, tag="ost", name="st")
                for half in range(2):
                    sc = pr * 2 + half
                    ps = pool.tile([128, 512], F32, tag="p1", name="ps3")
                    for h in range(R):
                        nc.tensor.matmul(
                            ps,
                            wo_sb[:, h, et * 128 : (et + 1) * 128],
                            outT[:, h, sc * 512 : (sc + 1) * 512],
                            start=(h == 0), stop=(h == R - 1),
                        )
                    cfg["evac"](st[:, half * 512 : (half + 1) * 512], ps)
                # alternate output triggers across two queues so the final
                # transfers don't serialize behind one trigger stream
                eng = nc.gpsimd if (et + pr) % 2 == 0 else nc.scalar
                eng.dma_start(
                    otd[et * 128 : (et + 1) * 128,
                        pr * 1024 : (pr + 1) * 1024],
                    st,
                )

            side = []

            def pump(n):
                for _ in range(n):
                    if side:
                        side.pop(0)()

            # ---------- upfront projections (interleaved K/V per chunk to
            # match x chunk arrival order; chunk 3's K/V become the first
            # side units so iter0 starts as soon as wq(h0) lands) ----------
            for sc in range(3):
                unit_proj("k", sc)
                unit_proj("v", sc, defer_tr=True)
            unit_proj("q", 0, 0)
            unit_proj("q", 1, 0)
            flush_tr()
            # wo is only needed once output-projection side units start
            # (~100us in): trigger from the now-quiet Sync queue so its 2MiB
            # doesn't compete with the startup x loads
            nc.sync.dma_start(wo_sb, wo[:, :, :])

            # remaining projections stream in as side work, ordered by when
            # the attention iterations consume them: K/V(s3) are needed from
            # iter0 tt=12 on; pr0 iters need Q(h,0),(h,1); pr1 (h,2),(h,3)
            side.append(lambda: unit_proj("k", 3))
            side.append(lambda: unit_proj("v", 3))
            for h in range(1, R):
                side.append(lambda h=h: unit_proj("q", 0, h))
                side.append(lambda h=h: unit_proj("q", 1, h))
            for h in range(R):
                side.append(lambda h=h: unit_proj("q", 2, h))
                side.append(lambda h=h: unit_proj("q", 3, h))

            # ---------- attention + interleaved side units ----------
            def mm_scores(pss, h, q0, tt):
                kslice = KT[:, tt * 128 : (tt + 1) * 128]
                for hf in range(2):
                    nc.tensor.matmul(
                        pss[:, hf * 512 : (hf + 1) * 512],
                        kslice,
                        QT[:, h, q0 + hf * 512 : q0 + (hf + 1) * 512],
                        start=True, stop=True,
                    )

            iters = [(pr, h) for pr in range(NPAIR) for h in range(R)]
            deferred = []  # (slot, closure): recip/mult of the PREVIOUS iter

            def flush_deferred(slot):
                while deferred and deferred[0][0] <= slot:
                    deferred.pop(0)[1]()

            for it, (pr, h) in enumerate(iters):
                cfg["evac"] = (
                    nc.scalar.copy if it >= 5 else nc.vector.tensor_copy
                )
                q0 = pr * 1024
                out_ps = psA.tile([128, 1024], F32, tag="av", name="out_ps")
                pss_tiles = [None, None]
                pss_tiles[0] = psS.tile([128, 1024], F32, tag="sc", name="pss")
                mm_scores(pss_tiles[0], h, q0, 0)
                acc_a = accp.tile([128, 1024], BF16, tag="acca", name="acc_a")
                acc_b = accp.tile([128, 1024], BF16, tag="accb", name="acc_b")
                for tt in range(ST):
                    pt = probs_pool.tile([128, 1024], BF16, tag="pb", name="pt")
                    nc.scalar.activation(pt, pss_tiles[tt % 2], Exp)
                    # keep independent PE work queued ahead of the
                    # exp-gated AV matmuls
                    if tt + 1 < ST:
                        pss_tiles[(tt + 1) % 2] = psS.tile(
                            [128, 1024], F32, tag="sc", name="pss"
                        )
                        mm_scores(pss_tiles[(tt + 1) % 2], h, q0, tt + 1)
                    for hf in range(2):
                        hs = slice(hf * 512, (hf + 1) * 512)
                        nc.tensor.matmul(
                            out_ps[:, hs], V[:, tt], pt[:, hs],
                            start=(tt == 0), stop=(tt == ST - 1),
                        )
                    # softmax denominators: bf16 elementwise accumulation
                    # on DVE (two 8-deep chains), off the PE entirely
                    if tt == 0:
                        nc.vector.tensor_copy(acc_a, pt)
                    elif tt == 1:
                        nc.vector.tensor_copy(acc_b, pt)
                    elif tt % 2 == 0:
                        nc.vector.tensor_tensor(acc_a, acc_a, pt, Add)
                    else:
                        nc.vector.tensor_tensor(acc_b, acc_b, pt, Add)
                    # previous iter's slow reciprocal runs HERE, mid-iter,
                    # where the in-order DVE queue has slack - never at an
                    # iteration boundary where it would gate probs recycling
                    flush_deferred(tt)
                    # iter0 must emit K/V(s3) before the attention matmuls
                    # that consume them (in-order PE queue: consumer-first
                    # would deadlock), and early enough to hide x3 landing
                    if (
                        tt in (7, 15)
                        or (it >= 4 and tt in (3, 11))
                        or (it == 0 and tt in (7, 9, 11))
                    ):
                        pump(1)
                # partition-reduce the two chain accumulators: 4 small
                # matmuls -> sums replicated across partitions
                sums_ps = psS.tile([128, 1024], F32, tag="sc", name="sums_ps")
                for ai, acc in enumerate((acc_a, acc_b)):
                    for hf in range(2):
                        hs = slice(hf * 512, (hf + 1) * 512)
                        nc.tensor.matmul(
                            sums_ps[:, hs], ones, acc[:, hs],
                            start=(ai == 0), stop=(ai == 1),
                        )
                av_sb = normp.tile([128, 1024], BF16, tag="a", name="av_sb")
                sums_sb = normp.tile([128, 1024], F32, tag="s", name="sums_sb")
                if it >= 5:
                    # Q projections are done by now: ACT has slack, DVE is
                    # the congested engine at iteration tails
                    nc.scalar.copy(av_sb, out_ps)
                    nc.scalar.copy(sums_sb, sums_ps)
                else:
                    nc.vector.tensor_copy(av_sb, out_ps)
                    nc.vector.tensor_copy(sums_sb, sums_ps)

                # the ~6.5us DVE reciprocal would starve probs recycling if
                # run whole: split into 4 chunks spread between next-iter
                # adds, with the normalize multiply trailing
                rc = normp.tile([128, 1024], BF16, tag="r", name="rc")

                def rchunk(c, sums_sb=sums_sb, rc=rc):
                    cs4 = slice(c * 256, (c + 1) * 256)
                    with nc.allow_low_precision(
                        reason="bf16 softmax scale, ~0.4% ok at 2e-2 gate"
                    ):
                        nc.vector.reciprocal(rc[:, cs4], sums_sb[:, cs4])

                def fmult(h=h, q0=q0, av_sb=av_sb, rc=rc):
                    # all-bf16 multiply on the otherwise-idle GpSimd engine
                    nc.gpsimd.tensor_tensor(
                        outT[:, h, q0 : q0 + 1024], av_sb, rc, Mult
                    )

                for c in range(4):
                    deferred.append((3 + 2 * c, lambda c=c: rchunk(c)))
                deferred.append((11, fmult))
                if it == 3:
                    # pr0 fully normalized soon: its output projection
                    # columns become available side work
                    for et in range(ET):
                        side.append(lambda et=et: unit_p3(et, 0))
            # final drain: release the attention PSUM banks and run the
            # remaining output-projection rows with deep buffering
            attn_stack.close()
            psD = ep(tc.tile_pool(name="psD", bufs=6, space="PSUM"))
            cfg["p3pool"] = psD
            cfg["evac"] = nc.vector.tensor_copy
            leftovers = list(side)
            side.clear()
            # interleave the last iteration's deferred normalize between the
            # first drain units so it never head-blocks their evacuations
            for i, fn in enumerate(leftovers):
                fn()
                flush_deferred(3 + 2 * i)
            flush_deferred(ST)
            for et in range(ET):
                unit_p3(et, 1)

    _split_multi_waits(nc)
    return nc


def _prepare(x, Wq, bq, Wk, bk, Wv, bv, Wo, bo):
    """Host-side sharding: build per-core input maps (bf16 operands)."""
    import ml_dtypes

    bf16 = ml_dtypes.bfloat16
    x = np.asarray(x, dtype=np.float32)
    Wq = np.asarray(Wq, dtype=np.float32)
    bq = np.asarray(bq, dtype=np.float32)
    Wk = np.asarray(Wk, dtype=np.float32)
    bk = np.asarray(bk, dtype=np.float32)
    Wv = np.asarray(Wv, dtype=np.float32)
    bv = np.asarray(bv, dtype=np.float32)
    Wo = np.asarray(Wo, dtype=np.float32)

    isd = np.float32(1.0 / np.sqrt(D))

    # pre-tile everything into the partition-major SBUF images the kernel
    # DMAs verbatim: contiguous per-partition rows = minimal descriptors
    def tile_w(w):  # [E, M] -> [128, ET, M]
        m = w.shape[1]
        return np.ascontiguousarray(
            w.reshape(ET, 128, m).transpose(1, 0, 2)
        ).astype(bf16)

    xds = [
        np.ascontiguousarray(
            x[b].T.reshape(ET, 128, SC, 512).transpose(1, 2, 0, 3)
        ).astype(bf16)
        for b in range(B)
    ]
    # wq is head-major [128, H, ET, D] so the kernel can pull head 0 early
    Wq_s = np.ascontiguousarray(
        (Wq * isd).reshape(ET, 128, H, D).transpose(1, 2, 0, 3)
    ).astype(bf16)
    Wk_s = tile_w(Wk)
    Wv_s = tile_w(Wv)
    Wo_t = np.ascontiguousarray(
        Wo.reshape(G, R, 128, E).transpose(2, 0, 1, 3)
    ).astype(bf16)  # [128, G, R, E]
    in_maps = []
    for core in range(8):
        b, g = divmod(core, G)
        in_maps.append({
            "xd": xds[b],
            "wq": np.ascontiguousarray(Wq_s[:, g * R : (g + 1) * R]),
            "wk": np.ascontiguousarray(Wk_s[:, :, g * D : (g + 1) * D]),
            "wv": np.ascontiguousarray(Wv_s[:, :, g * D : (g + 1) * D]),
            "wo": np.ascontiguousarray(Wo_t[:, g]),
            "bqv": bq[g * R * D : (g + 1) * R * D] * isd,
            "bkv": bk[g * D : (g + 1) * D],
            "bvv": bv[g * D : (g + 1) * D],
        })
    return in_maps


def _gather(results, bo):
    bo = np.asarray(bo, dtype=np.float32)
    out = np.empty((B, S, E), dtype=np.float32)
    for b in range(B):
        acc = results[b * G]["ot"].astype(np.float32)
        for g in range(1, G):
            acc += results[b * G + g]["ot"].astype(np.float32)
        out[b] = acc.T + bo
    return out


def kernel(x, Wq, bq, Wk, bk, Wv, bv, Wo, bo):
    from concourse.bass_utils import run_bass_kernel_spmd

    if "nc" not in _cache:
        _cache["nc"] = _build_program()
    nc = _cache["nc"]
    in_maps = _prepare(x, Wq, bq, Wk, bk, Wv, bv, Wo, bo)
    res = run_bass_kernel_spmd(nc, in_maps, core_ids=list(range(8)))
    return _gather(res.results, bo)
